# revision 27
# baseline (speedup 1.0000x reference)
"""Trainium2 Bass kernel for FCGF point-attention + FC head (segment softmax pool).

Pipeline per the nn.Module reference:
  att = relu(bn1(x @ w1.T + b1)) ; att = relu(bn2(att @ w2.T + b2))   [N, K]
  per-segment softmax over points, weighted pool of x -> [B, K, C]
  flatten -> FC -> bn3 -> L2 normalize -> [B, 256]

Distribution: data-parallel over point clouds; 16 segments paired onto 8 cores
(greedy balanced, 2 per core). Single fused pass per core: conv1 is software-
pipelined one tile ahead of the conv2/exp/pool block loop, both k-halves of
conv2 write one 2-bank PSUM tile so a single wide ACT computes exp over
[128, 1024], and both k-half pool accumulators run concurrently. The BN2 bias
cancels inside the softmax ratio except through the relu clamp, which becomes
max(exp(z), exp(-b2')) (one tensor_tensor max, no multiply). After pooling a
single bf16 AllToAll reshards the [16, 32768] pooled matrix by contraction
chunk; rows are written k-major so the FC needs no on-device transposes (the
lhsT [128, 16] tiles are strided column views of the resharded buffer).
Per-core FC partials are summed on host with the tiny BN3 + L2 epilogue.

A dummy AllGather is emitted before the TileContext so its doorbell is the
first gpsimd instruction: the ~50us ncfw communicator bootstrap runs
concurrently with the compute instead of gating the real AllToAll.
"""

import sys

if "/opt/trn_rl_repo" not in sys.path:
    sys.path.insert(0, "/opt/trn_rl_repo")

import numpy as np
import ml_dtypes

import concourse.bacc as bacc
import concourse.mybir as mybir
import concourse.tile as tile
from concourse.bass_utils import run_bass_kernel_spmd

B, N, C_IN, FC0, K = 16, 32768, 32, 256, 1024
BN_EPS = 1e-5
N_CORES = 8
TILE = 512
BF16 = ml_dtypes.bfloat16

LAST_RESULT = None  # test harness reads exec_time_ns from here
_PROGRAM_CACHE = {}


def _build_program(ntiles):
    """One SPMD program for all 8 cores; per-core behavior comes from data."""
    P = ntiles * TILE
    NB = ntiles * 4  # 128-point blocks
    dt = mybir.dt
    act = mybir.ActivationFunctionType
    alu = mybir.AluOpType
    nc = bacc.Bacc("TRN2", target_bir_lowering=False, debug=False,
                   num_devices=N_CORES)

    xT_in = nc.dram_tensor("xT", [2 * C_IN, P], dt.bfloat16, kind="ExternalInput")
    xab_in = nc.dram_tensor("xab", [128, NB * 66], dt.bfloat16,
                            kind="ExternalInput")
    w1T_in = nc.dram_tensor("w1T", [2 * C_IN, 128], dt.bfloat16, kind="ExternalInput")
    b1_in = nc.dram_tensor("b1c", [128, 2], dt.float32, kind="ExternalInput")
    w2T_in = nc.dram_tensor("w2T", [128, 2 * K], dt.float8e4, kind="ExternalInput")
    ie2_in = nc.dram_tensor("ie2r", [128, K], dt.bfloat16, kind="ExternalInput")
    fcw_in = nc.dram_tensor("fcwj", [128, 32 * FC0], dt.bfloat16,
                            kind="ExternalInput")
    id_in = nc.dram_tensor("ident", [66, 66], dt.float32, kind="ExternalInput")
    r_out = nc.dram_tensor("r", [B, FC0], dt.float32, kind="ExternalOutput")

    with tile.TileContext(nc) as tc:
        with (
            tc.tile_pool(name="const", bufs=1) as constp,
            tc.tile_pool(name="h1s", bufs=3) as h1s,
            tc.tile_pool(name="es", bufs=4) as es,
            tc.tile_pool(name="mis", bufs=2) as mis,
            tc.tile_pool(name="pacc", bufs=1, space="PSUM") as pacc,
            tc.tile_pool(name="dram", bufs=1, space="DRAM") as dram,
        ):
            # warm-up collective, first on the gpsimd queue: the doorbell
            # starts the ~65us ncfw communicator bootstrap so it overlaps the
            # compute phase instead of delaying the real AllToAll
            warm_in = dram.tile([1, 8], dt.float32, tag="warmin")
            warm_out = dram.tile([8, 8], dt.float32, tag="warmout")
            nc.gpsimd.collective_compute(
                "AllGather", alu.bypass,
                replica_groups=[list(range(N_CORES))],
                ins=[warm_in.opt()], outs=[warm_out.opt()],
            )

            # --- input loads: bulk on sync+gpsimd; scalar stays free for
            # relu/exp (a scalar-queue DMA backlog would stall the pipeline)
            w1T = constp.tile([2 * C_IN, 128], dt.bfloat16)
            nc.scalar.dma_start(w1T[:], w1T_in[:])
            b1c = constp.tile([128, 2], dt.float32)
            nc.scalar.dma_start(b1c[:], b1_in[:])

            xT = constp.tile([2 * C_IN, P], dt.bfloat16)
            nc.sync.dma_start(xT[:, 0:TILE], xT_in[:, 0:TILE])
            w2f8 = constp.tile([128, 2 * K], dt.float8e4)
            nc.sync.dma_start(w2f8[:], w2T_in[:])
            ie2r = constp.tile([128, K], dt.bfloat16)
            nc.sync.dma_start(ie2r[:], ie2_in[:])
            nc.sync.dma_start(xT[:, TILE:P], xT_in[:, TILE:P])
            identt = constp.tile([66, 66], dt.float32)
            nc.sync.dma_start(identt[:], id_in[:])

            xab = constp.tile([128, NB * 66], dt.bfloat16)
            nc.gpsimd.dma_start(xab[:, 0:4 * 66], xab_in[:, 0:4 * 66])
            nc.gpsimd.dma_start(xab[:, 4 * 66:NB * 66], xab_in[:, 4 * 66:NB * 66])
            fcw = constp.tile([128, 32 * FC0], dt.bfloat16)

            pacc0 = pacc.tile([66, TILE], dt.float32, tag="pacc0")
            pacc1 = pacc.tile([66, TILE], dt.float32, tag="pacc1")

            h1t = {}
            with (
                tc.tile_pool(name="ph1", bufs=2, space="PSUM") as ph1,
                tc.tile_pool(name="pap", bufs=2, space="PSUM") as pap,
            ):
                def conv1(t):
                    tsl = slice(t * TILE, (t + 1) * TILE)
                    # fp8 h1, both fc0-halves side by side for DoubleRow
                    h1f = h1s.tile([128, 2 * TILE], dt.float8e4, tag="h1f")
                    for h in range(2):
                        hp = ph1.tile([128, TILE], dt.float32, tag="h1p")
                        nc.tensor.matmul(hp[:],
                                         w1T[h * C_IN:(h + 1) * C_IN, :],
                                         xT[h * C_IN:(h + 1) * C_IN, tsl],
                                         start=True, stop=True,
                                         tile_position=(h * C_IN, 0))
                        hsl = h1f[:, h * TILE:(h + 1) * TILE]
                        if t % 2 == 0:
                            nc.scalar.activation(hsl, hp[:], act.Relu,
                                                 bias=b1c[:, h:h + 1])
                        else:
                            nc.vector.tensor_scalar(hsl, hp[:],
                                                    b1c[:, h:h + 1],
                                                    0.0, alu.add, alu.max)
                    h1t[t] = h1f
                    if t == min(5, ntiles - 1):
                        # gate the bulk FC-weight load on mid-compute data so
                        # it doesn't steal HBM bandwidth from the ramp loads
                        # (the copy scribbles on fcw -> WAW edge orders the
                        # dma after it; the dma then overwrites everything)
                        nc.gpsimd.tensor_copy(fcw[0:1, 0:8], h1f[0:1, 0:8])
                        nc.gpsimd.dma_start(fcw[:], fcw_in[:])

                # main loop: conv1 pipelined one tile ahead; pool MMs two
                # blocks behind their conv2 so exp+max never stalls the PE
                conv1(0)
                pend = []  # (blk, e_tile) awaiting pool matmuls

                def pool_mms(blk, e_sb):
                    xsl = slice(blk * 66, (blk + 1) * 66)
                    nc.tensor.matmul(pacc0[:], xab[:, xsl], e_sb[:, 0:TILE],
                                     start=(blk == 0), stop=(blk == NB - 1),
                                     skip_group_check=True)
                    nc.tensor.matmul(pacc1[:], xab[:, xsl],
                                     e_sb[:, TILE:2 * TILE],
                                     start=(blk == 0), stop=(blk == NB - 1),
                                     skip_group_check=True)

                for t in range(ntiles):
                    for s in range(4):
                        blk = t * 4 + s
                        if s == 2 and t + 1 < ntiles:
                            conv1(t + 1)
                        ap2 = pap.tile([128, 2 * TILE], dt.float32, tag="ap2")
                        # DoubleRow fp8: one MM per k-half, contraction 256
                        # packed two-per-cell ([p][ko=2][free] 3D operands)
                        lhs3 = h1t[t][:].rearrange("p (ko s m) -> s p ko m",
                                                   ko=2, s=4, m=128)[s]
                        rhs3 = w2f8[:].rearrange("p (ko kq n) -> kq p ko n",
                                                 ko=2, kq=2, n=TILE)
                        for kq in range(2):
                            ksl = slice(kq * TILE, (kq + 1) * TILE)
                            nc.tensor.matmul(
                                ap2[:, ksl], lhs3, rhs3[kq],
                                start=True, stop=True,
                                perf_mode=mybir.MatmulPerfMode.DoubleRow)
                        e_sb = es.tile([128, 2 * TILE], dt.bfloat16, tag="e")
                        nc.scalar.activation(e_sb[:], ap2[:], act.Exp)
                        nc.vector.tensor_max(e_sb[:], e_sb[:], ie2r[:])
                        pend.append((blk, e_sb))
                        if len(pend) > 2:
                            pool_mms(*pend.pop(0))
                    del h1t[t]
                for item in pend:
                    pool_mms(*item)

            # normalize both halves; o2 is k-major on partitions (bf16).
            # kq0's divides run on vector, kq1's on scalar; the i-loops of
            # the two halves interleave so neither chain gates the transposes
            with tc.tile_pool(name="ptail", bufs=4, space="PSUM") as ptail:
                pool_sb0 = mis.tile([66, TILE], dt.float32, tag="poolsb")
                nc.vector.tensor_copy(pool_sb0[:], pacc0[:])
                pool_sb1 = mis.tile([66, TILE], dt.float32, tag="poolsb1")
                nc.scalar.activation(pool_sb1[:], pacc1[:], act.Copy)
                psbs = [pool_sb0, pool_sb1]
                o2 = [mis.tile([128, 256], dt.bfloat16, tag="o2", bufs=2,
                               name=f"o2t{q}") for q in range(2)]
                for i in range(4):
                    for kq in range(2):
                        ptp = ptail.tile([128, 66], dt.float32, tag="ptp")
                        nc.tensor.transpose(
                            ptp[:], psbs[kq][:, i * 128:(i + 1) * 128],
                            identt[:])
                        for s2 in range(2):
                            zr = mis.tile([128, 1], dt.float32, tag="zr",
                                          bufs=4)
                            nc.vector.reciprocal(
                                zr[:], ptp[:, 33 * s2 + 32:33 * s2 + 33])
                            osl = slice(i * 64 + s2 * 32,
                                        i * 64 + s2 * 32 + 32)
                            if kq == 0:
                                nc.vector.tensor_scalar_mul(
                                    o2[kq][:, osl],
                                    ptp[:, 33 * s2:33 * s2 + 32], zr[:])
                            else:
                                nc.scalar.activation(
                                    o2[kq][:, osl],
                                    ptp[:, 33 * s2:33 * s2 + 32],
                                    act.Copy, scale=zr[:])

                # sender-side k-major rows: element (i,ph,kq,kp,s,c) of o2
                # goes to row 2i+ph, col kq*4096 + kp*64 + s*32 + c
                out2_d = dram.tile([8, 8192], dt.bfloat16, tag="out2d")
                fcin_d = dram.tile([8, 8192], dt.bfloat16, tag="fcind")
                dst = out2_d[:].rearrange("(i ph) (kq kp sc) -> ph kq kp i sc",
                                          i=4, ph=2, kq=2, kp=64, sc=64)
                for kq in range(2):
                    src4 = o2[kq][:].rearrange("p (i sc) -> p i sc",
                                               i=4, sc=64)
                    eng = nc.sync if kq == 0 else nc.scalar
                    for ph in range(2):
                        eng.dma_start(dst[ph, kq],
                                      src4[ph * 64:(ph + 1) * 64])
                nc.gpsimd.collective_compute(
                    "AllToAll", alu.bypass,
                    replica_groups=[list(range(N_CORES))],
                    ins=[out2_d.opt()], outs=[fcin_d.opt()],
                )
                # receiver: partition p = (kq,kp) maps linearly to col p*64
                fcin = mis.tile([128, 512], dt.bfloat16, tag="fcin")
                nc.sync.dma_start(
                    fcin[:].rearrange("p (a sc) -> p a sc", a=8, sc=64),
                    fcin_d[:].rearrange("a (p sc) -> p a sc", p=128, sc=64))

                fc_ps = ptail.tile([B, FC0], dt.float32, tag="fcps", bufs=1)
                fcv = fcin[:].rearrange("p (asx c) -> p c asx", asx=16, c=32)
                for cc in range(32):
                    nc.tensor.matmul(fc_ps[:], fcv[:, cc],
                                     fcw[:, cc * FC0:(cc + 1) * FC0],
                                     start=(cc == 0), stop=(cc == 31),
                                     skip_group_check=True)
                r_sb = mis.tile([B, FC0], dt.float32, tag="rsb")
                nc.vector.tensor_copy(r_sb[:], fc_ps[:])
                nc.sync.dma_start(r_out[:], r_sb[:])

    nc.compile()
    return nc


def _segment_runs(length):
    """Contiguous [start, end) row-run per segment, mirroring
    jnp.repeat(arange(B), length, total_repeat_length=N)."""
    length = np.asarray(length, np.int64)
    seg = np.repeat(np.arange(B), np.maximum(length, 0))
    if len(seg) >= N:
        seg = seg[:N]
    else:
        seg = np.pad(seg, (0, N - len(seg)), constant_values=B - 1)
    runs = []
    for b in range(B):
        idx = np.nonzero(seg == b)[0]
        if len(idx):
            runs.append((int(idx[0]), int(idx[-1]) + 1))
        else:
            runs.append((0, 0))
    return runs


def _pair_segments(runs):
    """Greedy balanced pairing: largest with smallest."""
    sizes = np.array([e - s for s, e in runs])
    order = list(np.argsort(-sizes))
    pairs = [(int(order[i]), int(order[B - 1 - i])) for i in range(B // 2)]
    return pairs


def kernel(**inputs):
    global LAST_RESULT
    f32 = np.float32
    x = np.asarray(inputs["x"], f32)
    length = np.asarray(inputs["length"])
    w1 = np.asarray(inputs["w1"], f32); b1 = np.asarray(inputs["b1"], f32)
    g1 = np.asarray(inputs["g1"], f32); be1 = np.asarray(inputs["be1"], f32)
    m1 = np.asarray(inputs["m1"], f32); v1 = np.asarray(inputs["v1"], f32)
    w2 = np.asarray(inputs["w2"], f32); b2 = np.asarray(inputs["b2"], f32)
    g2 = np.asarray(inputs["g2"], f32); be2 = np.asarray(inputs["be2"], f32)
    m2 = np.asarray(inputs["m2"], f32); v2 = np.asarray(inputs["v2"], f32)
    fcw = np.asarray(inputs["fcw"], f32); fcb = np.asarray(inputs["fcb"], f32)
    g3 = np.asarray(inputs["g3"], f32); be3 = np.asarray(inputs["be3"], f32)
    m3 = np.asarray(inputs["m3"], f32); v3 = np.asarray(inputs["v3"], f32)

    # fold BN1/BN2 into the conv weights
    a1 = g1 / np.sqrt(v1 + BN_EPS)
    w1p = (a1[:, None] * w1).astype(f32)
    b1p = (a1 * (b1 - m1) + be1).astype(f32)
    a2 = g2 / np.sqrt(v2 + BN_EPS)
    w2p = (a2[:, None] * w2).astype(f32)
    b2p = (a2 * (b2 - m2) + be2).astype(f32)
    ie2 = np.exp(-b2p).astype(f32)  # relu clamp: e = max(exp(z), exp(-b2'))

    runs = _segment_runs(length)
    pairs = _pair_segments(runs)
    lenf = np.asarray(length, f32)
    max_pair = max(
        (runs[a][1] - runs[a][0]) + (runs[b][1] - runs[b][0]) for a, b in pairs
    )
    ntiles = max(1, -(-int(max_pair) // TILE))
    P = ntiles * TILE
    NB = ntiles * 4

    # shared parameter tensors; w1T stacked [2*32, 128] for PE row-strips
    w1Tfull = w1p.T.astype(BF16)                   # [32, 256]
    w1T = np.vstack([w1Tfull[:, 0:128], w1Tfull[:, 128:256]])  # [64, 128]
    b1c = b1p.reshape(2, 128).T.astype(f32).copy() # [128, 2]
    w2Tf = w2p.T.astype(f32)                       # [256, 1024]
    # DoubleRow packing: [ki, ko*K + n] = w2T[ko*128 + ki, n], fp8e4
    w2T = np.concatenate([w2Tf[0:128], w2Tf[128:256]], axis=1).astype(
        ml_dtypes.float8_e4m3)
    ie2r = np.broadcast_to(ie2, (128, K)).astype(BF16).copy()
    fcwT = fcw.T.astype(BF16)                      # [32768, 256]
    ident = np.eye(66, dtype=f32)

    in_maps = []
    for c, (sa, sb) in enumerate(pairs):
        (a0, a1e), (b0, b1e) = runs[sa], runs[sb]
        nA, nB_ = a1e - a0, b1e - b0
        xc = np.zeros((P, C_IN), f32)
        xc[:nA] = x[a0:a1e]
        xc[nA:nA + nB_] = x[b0:b1e]
        xab = np.zeros((P, 66), f32)
        if nA:
            xab[:nA, 0:32] = x[a0:a1e] / max(lenf[sa], 1e-30)
            xab[:nA, 32] = 1.0
        if nB_:
            xab[nA:nA + nB_, 33:65] = x[b0:b1e] / max(lenf[sb], 1e-30)
            xab[nA:nA + nB_, 65] = 1.0
        # [P, 66] -> [128, NB*66] block-packed for strided LDWEIGHTS views
        xabp = np.ascontiguousarray(
            xab.reshape(NB, 128, 66).transpose(1, 0, 2).reshape(128, NB * 66)
        ).astype(BF16)
        # FC contraction chunk, k-major rows: p=(kq,kp) -> global k, col c
        pidx = np.arange(128)
        kg = (pidx // 64) * 512 + c * 64 + (pidx % 64)           # [128]
        rows = kg[:, None] * 32 + np.arange(32)[None, :]         # [128, 32]
        fcwj = fcwT[rows].reshape(128, 32 * FC0)
        xTc = np.ascontiguousarray(xc.T).astype(BF16)
        in_maps.append({
            "xT": np.vstack([xTc, xTc]),
            "xab": xabp,
            "w1T": w1T, "b1c": b1c, "w2T": w2T, "ie2r": ie2r,
            "fcwj": np.ascontiguousarray(fcwj),
            "ident": ident,
        })

    if ntiles not in _PROGRAM_CACHE:
        _PROGRAM_CACHE[ntiles] = _build_program(ntiles)
    nc = _PROGRAM_CACHE[ntiles]

    res = run_bass_kernel_spmd(nc, in_maps, list(range(N_CORES)))
    LAST_RESULT = res

    r = np.zeros((B, FC0), f32)
    for c in range(N_CORES):
        r += res.results[c]["r"]
    r += fcb
    a3 = g3 / np.sqrt(v3 + BN_EPS)
    r = (r - m3) * a3 + be3
    r = r / np.maximum(np.linalg.norm(r, axis=1, keepdims=True), 1e-12)

    # rows are in (core, pair-slot) order; map back to segment order
    out = np.empty((B, FC0), f32)
    for c, (sa, sb) in enumerate(pairs):
        out[sa] = r[2 * c]
        out[sb] = r[2 * c + 1]
    return out.astype(np.float32)


# revision 32
# speedup vs baseline: 1.3692x; 1.3692x over previous
"""Trainium2 Bass kernel for FCGF point-attention + FC head (segment softmax pool).

Pipeline per the nn.Module reference:
  att = relu(bn1(x @ w1.T + b1)) ; att = relu(bn2(att @ w2.T + b2))   [N, K]
  per-segment softmax over points, weighted pool of x -> [B, K, C]
  flatten -> FC -> bn3 -> L2 normalize -> [B, 256]

Distribution: data-parallel over point clouds; 16 segments paired onto 8 cores
(greedy balanced, 2 per core). Single fused pass per core: conv1 is software-
pipelined one tile ahead of the conv2/exp/pool block loop, both k-halves of
conv2 write one 2-bank PSUM tile so a single wide ACT computes exp over
[128, 1024], and both k-half pool accumulators run concurrently. The BN2 bias
cancels inside the softmax ratio except through the relu clamp, which becomes
max(exp(z), exp(-b2')) (one tensor_tensor max, no multiply). After pooling a
single bf16 AllToAll reshards the [16, 32768] pooled matrix by contraction
chunk; rows are written k-major so the FC needs no on-device transposes (the
lhsT [128, 16] tiles are strided column views of the resharded buffer).
Per-core FC partials are summed on host with the tiny BN3 + L2 epilogue.

A dummy AllGather is emitted before the TileContext so its doorbell is the
first gpsimd instruction: the ~50us ncfw communicator bootstrap runs
concurrently with the compute instead of gating the real AllToAll.
"""

import sys

if "/opt/trn_rl_repo" not in sys.path:
    sys.path.insert(0, "/opt/trn_rl_repo")

import numpy as np
import ml_dtypes

import concourse.bacc as bacc
import concourse.mybir as mybir
import concourse.tile as tile
from concourse.bass_utils import run_bass_kernel_spmd

B, N, C_IN, FC0, K = 16, 32768, 32, 256, 1024
BN_EPS = 1e-5
N_CORES = 8
TILE = 512
BF16 = ml_dtypes.bfloat16

LAST_RESULT = None  # test harness reads exec_time_ns from here
_PROGRAM_CACHE = {}


def _build_program(ntiles):
    """One SPMD program for all 8 cores; per-core behavior comes from data."""
    P = ntiles * TILE
    NB = ntiles * 4  # 128-point blocks
    dt = mybir.dt
    act = mybir.ActivationFunctionType
    alu = mybir.AluOpType
    nc = bacc.Bacc("TRN2", target_bir_lowering=False, debug=False,
                   num_devices=N_CORES)

    xT_in = nc.dram_tensor("xT", [2 * C_IN, P], dt.bfloat16, kind="ExternalInput")
    xab_in = nc.dram_tensor("xab", [128, NB * 66], dt.bfloat16,
                            kind="ExternalInput")
    w1T_in = nc.dram_tensor("w1T", [2 * C_IN, 128], dt.bfloat16, kind="ExternalInput")
    b1_in = nc.dram_tensor("b1c", [128, 2], dt.float32, kind="ExternalInput")
    w2T_in = nc.dram_tensor("w2T", [128, 2 * K], dt.float8e4, kind="ExternalInput")
    ie2_in = nc.dram_tensor("ie2r", [128, K], dt.bfloat16, kind="ExternalInput")
    fcw_in = nc.dram_tensor("fcwj", [128, 32 * FC0], dt.bfloat16,
                            kind="ExternalInput")
    id_in = nc.dram_tensor("ident", [66, 66], dt.float32, kind="ExternalInput")
    r_out = nc.dram_tensor("r", [B, FC0], dt.float32, kind="ExternalOutput")

    with tile.TileContext(nc) as tc:
        with (
            tc.tile_pool(name="const", bufs=1) as constp,
            tc.tile_pool(name="h1s", bufs=3) as h1s,
            tc.tile_pool(name="es", bufs=4) as es,
            tc.tile_pool(name="mis", bufs=2) as mis,
            tc.tile_pool(name="pacc", bufs=1, space="PSUM") as pacc,
            tc.tile_pool(name="dram", bufs=1, space="DRAM") as dram,
        ):
            # warm-up collective, first on the gpsimd queue: the doorbell
            # starts the ~65us ncfw communicator bootstrap so it overlaps the
            # compute phase instead of delaying the real AllToAll
            warm_in = dram.tile([1, 8], dt.float32, tag="warmin")
            warm_out = dram.tile([8, 8], dt.float32, tag="warmout")
            nc.gpsimd.collective_compute(
                "AllGather", alu.bypass,
                replica_groups=[list(range(N_CORES))],
                ins=[warm_in.opt()], outs=[warm_out.opt()],
            )

            # --- input loads: bulk on sync+gpsimd; scalar stays free for
            # relu/exp (a scalar-queue DMA backlog would stall the pipeline)
            w1T = constp.tile([2 * C_IN, 128], dt.bfloat16)
            nc.scalar.dma_start(w1T[:], w1T_in[:])
            b1c = constp.tile([128, 2], dt.float32)
            nc.scalar.dma_start(b1c[:], b1_in[:])

            w2f8 = constp.tile([128, 2 * K], dt.float8e4)
            nc.sync.dma_start(w2f8[:], w2T_in[:])
            xT = constp.tile([2 * C_IN, P], dt.bfloat16)
            nc.sync.dma_start(xT[:, 0:TILE], xT_in[:, 0:TILE])
            ie2r = constp.tile([128, K], dt.bfloat16)
            nc.sync.dma_start(ie2r[:], ie2_in[:])
            nc.sync.dma_start(xT[:, TILE:P], xT_in[:, TILE:P])
            identt = constp.tile([66, 66], dt.float32)
            nc.sync.dma_start(identt[:], id_in[:])

            xab = constp.tile([128, NB * 66], dt.bfloat16)
            nc.gpsimd.dma_start(xab[:, 0:4 * 66], xab_in[:, 0:4 * 66])
            nc.gpsimd.dma_start(xab[:, 4 * 66:NB * 66], xab_in[:, 4 * 66:NB * 66])
            fcw = constp.tile([128, 32 * FC0], dt.bfloat16)

            pacc0 = pacc.tile([66, TILE], dt.float32, tag="pacc0")
            pacc1 = pacc.tile([66, TILE], dt.float32, tag="pacc1")

            h1t = {}
            with (
                tc.tile_pool(name="ph1", bufs=2, space="PSUM") as ph1,
                tc.tile_pool(name="pap", bufs=2, space="PSUM") as pap,
            ):
                def conv1(t):
                    tsl = slice(t * TILE, (t + 1) * TILE)
                    # fp8 h1, both fc0-halves side by side for DoubleRow
                    h1f = h1s.tile([128, 2 * TILE], dt.float8e4, tag="h1f")
                    for h in range(2):
                        hp = ph1.tile([128, TILE], dt.float32, tag="h1p")
                        nc.tensor.matmul(hp[:],
                                         w1T[h * C_IN:(h + 1) * C_IN, :],
                                         xT[h * C_IN:(h + 1) * C_IN, tsl],
                                         start=True, stop=True,
                                         tile_position=(h * C_IN, 0))
                        hsl = h1f[:, h * TILE:(h + 1) * TILE]
                        if t % 2 == 0:
                            nc.scalar.activation(hsl, hp[:], act.Relu,
                                                 bias=b1c[:, h:h + 1])
                        else:
                            nc.vector.tensor_scalar(hsl, hp[:],
                                                    b1c[:, h:h + 1],
                                                    0.0, alu.add, alu.max)
                    h1t[t] = h1f
                    if t == min(6, ntiles - 1):
                        # gate the bulk FC-weight load on mid-compute data so
                        # it doesn't steal HBM bandwidth from the ramp loads
                        # (the copy scribbles on fcw -> WAW edge orders the
                        # dma after it; the dma then overwrites everything)
                        nc.gpsimd.tensor_copy(fcw[0:1, 0:8], h1f[0:1, 0:8])
                        nc.gpsimd.dma_start(fcw[:], fcw_in[:])

                # main loop: conv1 pipelined one tile ahead; pool MMs two
                # blocks behind their conv2 so exp+max never stalls the PE
                conv1(0)
                pend = []  # (blk, e_tile) awaiting pool matmuls

                def pool_mms(blk, e_sb):
                    xsl = slice(blk * 66, (blk + 1) * 66)
                    nc.tensor.matmul(pacc0[:], xab[:, xsl], e_sb[:, 0:TILE],
                                     start=(blk == 0), stop=(blk == NB - 1),
                                     skip_group_check=True)
                    nc.tensor.matmul(pacc1[:], xab[:, xsl],
                                     e_sb[:, TILE:2 * TILE],
                                     start=(blk == 0), stop=(blk == NB - 1),
                                     skip_group_check=True)

                for t in range(ntiles):
                    for s in range(4):
                        blk = t * 4 + s
                        if s == 2 and t + 1 < ntiles:
                            conv1(t + 1)
                        ap2 = pap.tile([128, 2 * TILE], dt.float32, tag="ap2")
                        # DoubleRow fp8: one MM per k-half, contraction 256
                        # packed two-per-cell ([p][ko=2][free] 3D operands)
                        lhs3 = h1t[t][:].rearrange("p (ko s m) -> s p ko m",
                                                   ko=2, s=4, m=128)[s]
                        rhs3 = w2f8[:].rearrange("p (ko kq n) -> kq p ko n",
                                                 ko=2, kq=2, n=TILE)
                        for kq in range(2):
                            ksl = slice(kq * TILE, (kq + 1) * TILE)
                            nc.tensor.matmul(
                                ap2[:, ksl], lhs3, rhs3[kq],
                                start=True, stop=True,
                                perf_mode=mybir.MatmulPerfMode.DoubleRow)
                        e_sb = es.tile([128, 2 * TILE], dt.bfloat16, tag="e")
                        nc.scalar.activation(e_sb[:], ap2[:], act.Exp)
                        nc.vector.tensor_max(e_sb[:], e_sb[:], ie2r[:])
                        pend.append((blk, e_sb))
                        if len(pend) > 2:
                            pool_mms(*pend.pop(0))
                    del h1t[t]
                for item in pend:
                    pool_mms(*item)

            # normalize both halves; o2 is k-major on partitions (bf16).
            # kq0's divides run on vector, kq1's on scalar; the i-loops of
            # the two halves interleave so neither chain gates the transposes
            with tc.tile_pool(name="ptail", bufs=4, space="PSUM") as ptail:
                pool_sb0 = mis.tile([66, TILE], dt.float32, tag="poolsb")
                nc.vector.tensor_copy(pool_sb0[:], pacc0[:])
                pool_sb1 = mis.tile([66, TILE], dt.float32, tag="poolsb1")
                nc.scalar.activation(pool_sb1[:], pacc1[:], act.Copy)
                psbs = [pool_sb0, pool_sb1]
                o2 = [mis.tile([128, 256], dt.bfloat16, tag="o2", bufs=2,
                               name=f"o2t{q}") for q in range(2)]
                for i in range(4):
                    for kq in range(2):
                        ptp = ptail.tile([128, 66], dt.float32, tag="ptp")
                        nc.tensor.transpose(
                            ptp[:], psbs[kq][:, i * 128:(i + 1) * 128],
                            identt[:])
                        # both Z columns (cols 32 and 65) in one stride-33 op
                        zr = mis.tile([128, 2], dt.float32, tag="zr", bufs=4)
                        nc.vector.reciprocal(
                            zr[:], ptp[:].rearrange("p (s z) -> p s z",
                                                    s=2, z=33)[:, :, 32])
                        for s2 in range(2):
                            osl = slice(i * 64 + s2 * 32,
                                        i * 64 + s2 * 32 + 32)
                            if kq == 0:
                                nc.vector.tensor_scalar_mul(
                                    o2[kq][:, osl],
                                    ptp[:, 33 * s2:33 * s2 + 32],
                                    zr[:, s2:s2 + 1])
                            else:
                                nc.scalar.activation(
                                    o2[kq][:, osl],
                                    ptp[:, 33 * s2:33 * s2 + 32],
                                    act.Copy, scale=zr[:, s2:s2 + 1])

                # sender-side k-major rows: element (i,ph,kq,kp,s,c) of o2
                # goes to row 2i+ph, col kq*4096 + kp*64 + s*32 + c
                out2_d = dram.tile([8, 8192], dt.bfloat16, tag="out2d")
                fcin_d = dram.tile([8, 8192], dt.bfloat16, tag="fcind")
                dst = out2_d[:].rearrange("(i ph) (kq kp sc) -> ph kq kp i sc",
                                          i=4, ph=2, kq=2, kp=64, sc=64)
                for kq in range(2):
                    src4 = o2[kq][:].rearrange("p (i sc) -> p i sc",
                                               i=4, sc=64)
                    eng = nc.sync if kq == 0 else nc.scalar
                    for ph in range(2):
                        eng.dma_start(dst[ph, kq],
                                      src4[ph * 64:(ph + 1) * 64])
                nc.gpsimd.collective_compute(
                    "AllToAll", alu.bypass,
                    replica_groups=[list(range(N_CORES))],
                    ins=[out2_d.opt()], outs=[fcin_d.opt()],
                )
                # receiver: partition p = (kq,kp) maps linearly to col p*64;
                # split across the two HWDGE queues by partition half
                fcin = mis.tile([128, 512], dt.bfloat16, tag="fcin")
                for half, eng in ((0, nc.sync), (1, nc.scalar)):
                    eng.dma_start(
                        fcin[half * 64:(half + 1) * 64, :].rearrange(
                            "p (a sc) -> p a sc", a=8, sc=64),
                        fcin_d[:, half * 4096:(half + 1) * 4096].rearrange(
                            "a (p sc) -> p a sc", p=64, sc=64))

                fc_ps = ptail.tile([B, FC0], dt.float32, tag="fcps", bufs=1)
                fcv = fcin[:].rearrange("p (asx c) -> p c asx", asx=16, c=32)
                for cc in range(32):
                    nc.tensor.matmul(fc_ps[:], fcv[:, cc],
                                     fcw[:, cc * FC0:(cc + 1) * FC0],
                                     start=(cc == 0), stop=(cc == 31),
                                     skip_group_check=True)
                r_sb = mis.tile([B, FC0], dt.float32, tag="rsb")
                nc.vector.tensor_copy(r_sb[:], fc_ps[:])
                nc.sync.dma_start(r_out[:], r_sb[:])

    nc.compile()
    return nc


def _segment_runs(length):
    """Contiguous [start, end) row-run per segment, mirroring
    jnp.repeat(arange(B), length, total_repeat_length=N)."""
    length = np.asarray(length, np.int64)
    seg = np.repeat(np.arange(B), np.maximum(length, 0))
    if len(seg) >= N:
        seg = seg[:N]
    else:
        seg = np.pad(seg, (0, N - len(seg)), constant_values=B - 1)
    runs = []
    for b in range(B):
        idx = np.nonzero(seg == b)[0]
        if len(idx):
            runs.append((int(idx[0]), int(idx[-1]) + 1))
        else:
            runs.append((0, 0))
    return runs


def _pair_segments(runs):
    """Greedy balanced pairing: largest with smallest."""
    sizes = np.array([e - s for s, e in runs])
    order = list(np.argsort(-sizes))
    pairs = [(int(order[i]), int(order[B - 1 - i])) for i in range(B // 2)]
    return pairs


def kernel(**inputs):
    global LAST_RESULT
    f32 = np.float32
    x = np.asarray(inputs["x"], f32)
    length = np.asarray(inputs["length"])
    w1 = np.asarray(inputs["w1"], f32); b1 = np.asarray(inputs["b1"], f32)
    g1 = np.asarray(inputs["g1"], f32); be1 = np.asarray(inputs["be1"], f32)
    m1 = np.asarray(inputs["m1"], f32); v1 = np.asarray(inputs["v1"], f32)
    w2 = np.asarray(inputs["w2"], f32); b2 = np.asarray(inputs["b2"], f32)
    g2 = np.asarray(inputs["g2"], f32); be2 = np.asarray(inputs["be2"], f32)
    m2 = np.asarray(inputs["m2"], f32); v2 = np.asarray(inputs["v2"], f32)
    fcw = np.asarray(inputs["fcw"], f32); fcb = np.asarray(inputs["fcb"], f32)
    g3 = np.asarray(inputs["g3"], f32); be3 = np.asarray(inputs["be3"], f32)
    m3 = np.asarray(inputs["m3"], f32); v3 = np.asarray(inputs["v3"], f32)

    # fold BN1/BN2 into the conv weights
    a1 = g1 / np.sqrt(v1 + BN_EPS)
    w1p = (a1[:, None] * w1).astype(f32)
    b1p = (a1 * (b1 - m1) + be1).astype(f32)
    a2 = g2 / np.sqrt(v2 + BN_EPS)
    w2p = (a2[:, None] * w2).astype(f32)
    b2p = (a2 * (b2 - m2) + be2).astype(f32)
    ie2 = np.exp(-b2p).astype(f32)  # relu clamp: e = max(exp(z), exp(-b2'))

    runs = _segment_runs(length)
    pairs = _pair_segments(runs)
    lenf = np.asarray(length, f32)
    max_pair = max(
        (runs[a][1] - runs[a][0]) + (runs[b][1] - runs[b][0]) for a, b in pairs
    )
    ntiles = max(1, -(-int(max_pair) // TILE))
    P = ntiles * TILE
    NB = ntiles * 4

    # shared parameter tensors; w1T stacked [2*32, 128] for PE row-strips
    w1Tfull = w1p.T.astype(BF16)                   # [32, 256]
    w1T = np.vstack([w1Tfull[:, 0:128], w1Tfull[:, 128:256]])  # [64, 128]
    b1c = b1p.reshape(2, 128).T.astype(f32).copy() # [128, 2]
    w2Tf = w2p.T.astype(f32)                       # [256, 1024]
    # DoubleRow packing: [ki, ko*K + n] = w2T[ko*128 + ki, n], fp8e4
    w2T = np.concatenate([w2Tf[0:128], w2Tf[128:256]], axis=1).astype(
        ml_dtypes.float8_e4m3)
    ie2r = np.broadcast_to(ie2, (128, K)).astype(BF16).copy()
    fcwT = fcw.T.astype(BF16)                      # [32768, 256]
    ident = np.eye(66, dtype=f32)

    in_maps = []
    for c, (sa, sb) in enumerate(pairs):
        (a0, a1e), (b0, b1e) = runs[sa], runs[sb]
        nA, nB_ = a1e - a0, b1e - b0
        xc = np.zeros((P, C_IN), f32)
        xc[:nA] = x[a0:a1e]
        xc[nA:nA + nB_] = x[b0:b1e]
        xab = np.zeros((P, 66), f32)
        if nA:
            xab[:nA, 0:32] = x[a0:a1e] / max(lenf[sa], 1e-30)
            xab[:nA, 32] = 1.0
        if nB_:
            xab[nA:nA + nB_, 33:65] = x[b0:b1e] / max(lenf[sb], 1e-30)
            xab[nA:nA + nB_, 65] = 1.0
        # [P, 66] -> [128, NB*66] block-packed for strided LDWEIGHTS views
        xabp = np.ascontiguousarray(
            xab.reshape(NB, 128, 66).transpose(1, 0, 2).reshape(128, NB * 66)
        ).astype(BF16)
        # FC contraction chunk, k-major rows: p=(kq,kp) -> global k, col c
        pidx = np.arange(128)
        kg = (pidx // 64) * 512 + c * 64 + (pidx % 64)           # [128]
        rows = kg[:, None] * 32 + np.arange(32)[None, :]         # [128, 32]
        fcwj = fcwT[rows].reshape(128, 32 * FC0)
        xTc = np.ascontiguousarray(xc.T).astype(BF16)
        in_maps.append({
            "xT": np.vstack([xTc, xTc]),
            "xab": xabp,
            "w1T": w1T, "b1c": b1c, "w2T": w2T, "ie2r": ie2r,
            "fcwj": np.ascontiguousarray(fcwj),
            "ident": ident,
        })

    if ntiles not in _PROGRAM_CACHE:
        _PROGRAM_CACHE[ntiles] = _build_program(ntiles)
    nc = _PROGRAM_CACHE[ntiles]

    res = run_bass_kernel_spmd(nc, in_maps, list(range(N_CORES)))
    LAST_RESULT = res

    r = np.zeros((B, FC0), f32)
    for c in range(N_CORES):
        r += res.results[c]["r"]
    r += fcb
    a3 = g3 / np.sqrt(v3 + BN_EPS)
    r = (r - m3) * a3 + be3
    r = r / np.maximum(np.linalg.norm(r, axis=1, keepdims=True), 1e-12)

    # rows are in (core, pair-slot) order; map back to segment order
    out = np.empty((B, FC0), f32)
    for c, (sa, sb) in enumerate(pairs):
        out[sa] = r[2 * c]
        out[sb] = r[2 * c + 1]
    return out.astype(np.float32)
